# revision 24
# baseline (speedup 1.0000x reference)
"""nn_MultiHeadAttention (B=2, S=2048, D=2048, H=16) on 8 NeuronCores.

The reference module splits heads with a plain reshape (no transpose):
    Q = (x @ Wq.T).reshape(B, H, S, Dh)
so head h attends over ROWS [128h, 128h+128) of Qmat = x @ Wq.T, with
attention position s = 16a + r mapping to (row 128h + a, feature slice
[128r, 128r+128)).  The merge DOES transpose (standard), so
    y = sum_h outh @ Wo[:, 128h:128h+128].T.

Sharding: core c handles batch b=c//4 and head-group g=c%4 (heads
4g..4g+3, i.e. Qmat/Kmat/Vmat rows [512g, 512g+512) of its batch).  Each
core computes those projection row-slices (against the FULL Wq/Wk/Wv,
streamed), causal attention in the scrambled index space, and a partial
output projection against its column slice of Wo.  The host sums the 4
partials per batch.

Attention layout per head (all on-chip, no transposes anywhere):
  k-tiles are r'-stripes {16a'+r' : a'}, q-blocks are 4 r-stripes wide.
  scoresT[k, q] tiles come out of the PE as [a', (ri, a)]; the causal mask
  within any (r', r) stripe pair is triangular in (a', a) (strict or
  inclusive depending on r' <= r), so 5 precomputed [128, 512] masks cover
  every tile.  Softmax denominators use a ones-column matmul on the PE and
  a K=1 ones-row matmul to broadcast 1/l across partitions.

Matmuls run as float32r (full-rate reduced-precision fp32) with fp32 PSUM
accumulation.
"""

import sys

try:
    import concourse.bass as bass
except ImportError:  # harness may not have the repo on PYTHONPATH
    for p in ("/root/.axon_site", "/root/.axon_site/_ro/trn_rl_repo",
              "/root/.axon_site/_ro/pypackages", "/opt/trn_rl_repo"):
        if p not in sys.path:
            sys.path.append(p)
    import concourse.bass as bass

import numpy as np

import concourse.mybir as mybir
import concourse.tile as tile
from concourse.bass_utils import run_bass_kernel_spmd

F32 = mybir.dt.float32
F32R = mybir.dt.float32r
BF16 = mybir.dt.bfloat16
DT = BF16  # on-chip matmul dtype
AF = mybir.ActivationFunctionType

B = 2
S = 2048
DM = 2048
H = 16
DH = 128
N_CORES = 8
HPC = 4                 # heads per core
DL = HPC * DH           # 512: per-core row/col slice width
P = 128
QB = 512                # q-block width = 4 r-stripes x 128 a
N_DM = DM // P          # 16 contraction tiles
NR = 16                 # r-stripes per head


def _split_multi_waits(nc):
    """This container's walrus rejects >1 sync-wait per instruction.
    Hoist extra waits onto same-engine NoOps inserted just before."""
    ctr = 0
    for f in nc.m.functions:
        for bb in f.blocks:
            insts = bb.instructions
            fixes = []
            for idx, inst in enumerate(insts):
                si = inst.sync_info
                ow = list(si.on_wait) if si and si.on_wait else []
                if len(ow) > 1:
                    fixes.append((idx, inst, ow, si))
            for idx, inst, ow, si in reversed(fixes):
                inst.sync_info = mybir.SyncInfo(on_wait=ow[-1:], on_update=si.on_update)
                for w in reversed(ow[:-1]):
                    ctr += 1
                    nop = mybir.InstNoOp(
                        name=f"I-waitsplit-{ctr}", engine=inst.engine, ins=[], outs=[]
                    )
                    nop.sync_info = mybir.SyncInfo(on_wait=[w], on_update=[])
                    nc.register_instruction(nop, overwrite=True)
                    insts.insert(idx, nop)
    return ctr


def _build_nc():
    nc = bass.Bass(target_bir_lowering=False)

    xs_d = nc.dram_tensor("xs", [DM, DL], DT, kind="ExternalInput")    # x[b,rows].T
    wqt_d = nc.dram_tensor("wqt", [DM, DM], DT, kind="ExternalInput")  # Wq.T (full)
    wkt_d = nc.dram_tensor("wkt", [DM, DM], DT, kind="ExternalInput")
    wvt_d = nc.dram_tensor("wvt", [DM, DM], DT, kind="ExternalInput")
    wot_d = nc.dram_tensor("wot", [DL, DM], DT, kind="ExternalInput")  # Wo[:,slice].T
    mask_d = nc.dram_tensor("maskc", [4, P, QB], DT, kind="ExternalInput")
    ones_d = nc.dram_tensor("ones", [P, P], DT, kind="ExternalInput")
    ident_d = nc.dram_tensor("ident", [P, P], DT, kind="ExternalInput")
    yt_d = nc.dram_tensor("yt", [DM, S], DT, kind="ExternalOutput")    # partial y[b].T

    yt_t3 = yt_d.rearrange("(o p) s -> p o s", p=P)

    with tile.TileContext(nc) as tc:
        with (
            tc.tile_pool(name="stage", bufs=4) as stage,
            tc.tile_pool(name="small", bufs=4) as small,
            tc.tile_pool(name="proj", bufs=HPC) as proj,
            tc.tile_pool(name="ps_s", bufs=3, space="PSUM") as ps_s,
            tc.tile_pool(name="ps_o", bufs=3, space="PSUM") as ps_o,
            tc.tile_pool(name="ps_l", bufs=2, space="PSUM") as ps_l,
            nc.allow_low_precision(reason="bf16 attention kernel"),
        ):
            # per-head projection tiles in [dh, a, r] layout, filled by phase A
            qt2 = [proj.tile([P, P, NR], DT, tag="qt2", name=f"qt2_{i}") for i in range(HPC)]
            kt2 = [proj.tile([P, P, NR], DT, tag="kt2", name=f"kt2_{i}") for i in range(HPC)]
            vt2 = [proj.tile([P, P, NR], DT, tag="vt2", name=f"vt2_{i}") for i in range(HPC)]

            # ---- phase A: projection row-slices straight into SBUF ----
            with (
                tc.tile_pool(name="xpool", bufs=1) as xpool,
                tc.tile_pool(name="wqk", bufs=3) as wqk,
            ):
                xs_t = xpool.tile([P, N_DM, DL], DT, tag="x")
                xs_t3 = xs_d.rearrange("(o p) s -> p o s", p=P)
                for i in range(4):
                    nc.gpsimd.dma_start(
                        xs_t[:, 4 * i:4 * (i + 1), :], xs_t3[:, 4 * i:4 * (i + 1), :]
                    )
                for w_d, dst in ((wvt_d, vt2), (wkt_d, kt2), (wqt_d, qt2)):
                    w_t3 = w_d.rearrange("(o p) d -> p o d", p=P)
                    for rt in range(NR):
                        w_t = wqk.tile([P, N_DM, P], DT, tag="wqk")
                        nc.sync.dma_start(w_t[:], w_t3[:, :, rt * P:(rt + 1) * P])
                        psum = ps_s.tile([P, QB], F32, tag="ps")
                        for dm in range(N_DM):
                            nc.tensor.matmul(
                                psum[:], lhsT=w_t[:, dm, :], rhs=xs_t[:, dm, :],
                                start=(dm == 0), stop=(dm == N_DM - 1),
                            )
                        # scatter into per-head [dh, a, r] tiles (r-strided)
                        for hl in range(HPC):
                            nc.any.tensor_copy(
                                dst[hl][:, :, rt], psum[:, hl * P:(hl + 1) * P]
                            )

            # ---- phase B: attention per head (scrambled index space) ----
            with (
                tc.tile_pool(name="bconst", bufs=1) as bconst,
                tc.tile_pool(name="hpool", bufs=2) as hpool,
                tc.tile_pool(name="atpool", bufs=6) as atpool,
                tc.tile_pool(name="attt", bufs=HPC) as attt_pool,
            ):
                ones_t = bconst.tile([P, P], DT, tag="ones")
                nc.sync.dma_start(ones_t[:], ones_d[:])
                mask_t = bconst.tile([P, 4, QB], DT, tag="mask")
                nc.sync.dma_start(mask_t[:], mask_d.rearrange("c p q -> p c q"))
                ident_t = bconst.tile([P, P], DT, tag="ident")
                nc.sync.dma_start(ident_t[:], ident_d[:])

                att_tiles = []
                for hl in range(HPC):
                    # vk: k-major V tiles via PE transpose, partition=(a'',r')
                    vk_h = hpool.tile([P, NR, P], DT, tag="v")    # [(a'' r'), m, dh]

                    def emit_vk(m, hl=hl, vk_h=vk_h):
                        ps_t = ps_o.tile([P, P], DT, tag="po")
                        nc.tensor.transpose(
                            ps_t[:], vt2[hl][:, 8 * m:8 * (m + 1), :], ident_t[:]
                        )
                        nc.any.tensor_copy(vk_h[:, m, :], ps_t[:])

                    att_h = attt_pool.tile([P, P, NR], DT, tag="attT")  # [dh, a, r]
                    att_tiles.append(att_h)
                    rcbs = []

                    for qb in range(4):
                        a0 = 32 * qb
                        nk = 4 * (qb + 1)   # k-octets 0..nk-1
                        for m in range(4 * qb, nk):
                            emit_vk(m)
                        psum_o = ps_o.tile([P, QB], F32, tag="po")
                        psum_l = ps_l.tile([P, QB], F32, tag="pl")
                        ats = [None] * nk

                        def emit_scores(m):
                            psum_s = ps_s.tile([P, QB], F32, tag="ps")
                            nc.tensor.matmul(
                                psum_s[:],
                                lhsT=kt2[hl][:, 8 * m:8 * (m + 1), :],
                                rhs=qt2[hl][:, a0:a0 + 32, :],
                                start=True, stop=True,
                            )
                            at = atpool.tile([P, QB], DT, tag="at")
                            nc.scalar.activation(at[:], psum_s[:], AF.Exp, scale=1.0 / DH)
                            if m >= 4 * qb:
                                nc.vector.tensor_mul(at[:], at[:], mask_t[:, m - 4 * qb, :])
                            ats[m] = at

                        def emit_ov(m):
                            nc.tensor.matmul(
                                psum_o[:],
                                lhsT=vk_h[:, m, :], rhs=ats[m][:],
                                start=(m == 0), stop=(m == nk - 1),
                            )
                            nc.tensor.matmul(
                                psum_l[:],
                                lhsT=ones_t[:, :], rhs=ats[m][:],
                                start=(m == 0), stop=(m == nk - 1),
                            )

                        DEPTH = 2
                        for m in range(nk):
                            emit_scores(m)
                            if m >= DEPTH:
                                emit_ov(m - DEPTH)
                        for m in range(nk - DEPTH, nk):
                            emit_ov(m)

                        # defer normalization: copy raw output, reciprocal lazily
                        nc.vector.tensor_copy(
                            att_h[:, a0:a0 + 32, :],
                            psum_o[:].rearrange("p (a r) -> p a r", a=32),
                        )
                        rcb = small.tile([P, QB], F32, tag=f"rcb{qb}")
                        rcbs.append(rcb)
                        nc.vector.reciprocal(rcb[:], psum_l[:])

                    # normalize the whole head once
                    for qb in range(4):
                        a0 = 32 * qb
                        nc.vector.tensor_mul(
                            att_h[:, a0:a0 + 32, :],
                            att_h[:, a0:a0 + 32, :],
                            rcbs[qb][:].rearrange("p (a r) -> p a r", a=32),
                        )

                # ---- phase C: partial output projection yT = WoT.T @ attT ----
                with tc.tile_pool(name="wop", bufs=1) as wop:
                    wot_t = wop.tile([P, HPC, DM], DT, tag="wo")
                    nc.sync.dma_start(
                        wot_t[:],
                        wot_d.rearrange("(hl p) d -> p hl d", p=P),
                    )
                    att_flat = [
                        t[:].rearrange("p a r -> p (a r)") for t in att_tiles
                    ]
                    for ot in range(N_DM):
                        for sb in range(4):
                            psum = ps_s.tile([P, QB], F32, tag="ps")
                            for hl in range(HPC):
                                nc.tensor.matmul(
                                    psum[:],
                                    lhsT=wot_t[:, hl, ot * P:(ot + 1) * P],
                                    rhs=att_flat[hl][:, sb * QB:(sb + 1) * QB],
                                    start=(hl == 0), stop=(hl == HPC - 1),
                                )
                            st = stage.tile([P, QB], DT, tag="ystage")
                            nc.any.tensor_copy(st[:], psum[:])
                            nc.gpsimd.dma_start(yt_t3[:, ot, sb * QB:(sb + 1) * QB], st[:])

    _split_multi_waits(nc)
    return nc


_NC = None


def _make_masks():
    # a-blocked causal masks for diagonal tiles, (a-outer, r-inner) order:
    # k partition index p = a''*16 + r';  q column index j = a_rel*16 + r
    # allow k <= q:  16*(8*mi + a'') + r'  <=  16*a_rel + r
    k_lin = (16 * np.arange(8)[:, None] + np.arange(NR)[None, :]).reshape(-1)   # 128
    q_lin = (16 * np.arange(32)[:, None] + np.arange(NR)[None, :]).reshape(-1)  # 512
    out = np.empty((4, P, QB), dtype=np.float32)
    for mi in range(4):
        out[mi] = ((k_lin[:, None] + 128 * mi) <= q_lin[None, :]).astype(np.float32)
    return out


def kernel(x, Wq, Wk, Wv, Wo, _want_trace=False, **_trace_kw):
    global _NC
    if _NC is None:
        _NC = _build_nc()
    nc = _NC

    import ml_dtypes
    bf16 = ml_dtypes.bfloat16

    x = np.asarray(x, dtype=np.float32)
    wqt = np.ascontiguousarray(np.asarray(Wq, dtype=np.float32).T).astype(bf16)
    wkt = np.ascontiguousarray(np.asarray(Wk, dtype=np.float32).T).astype(bf16)
    wvt = np.ascontiguousarray(np.asarray(Wv, dtype=np.float32).T).astype(bf16)
    Wo = np.asarray(Wo, dtype=np.float32)
    masks = _make_masks().astype(bf16)
    ones = np.ones((P, P), dtype=bf16)
    ident = np.eye(P, dtype=np.float32).astype(bf16)

    in_maps = []
    for c in range(N_CORES):
        b, g = divmod(c, HPC)
        sl = slice(g * DL, (g + 1) * DL)
        in_maps.append({
            "xs": np.ascontiguousarray(x[b, sl, :].T).astype(bf16),
            "wqt": wqt,
            "wkt": wkt,
            "wvt": wvt,
            "wot": np.ascontiguousarray(Wo[:, sl].T).astype(bf16),
            "maskc": masks,
            "ones": ones,
            "ident": ident,
        })

    res = run_bass_kernel_spmd(
        nc, in_maps, list(range(N_CORES)),
        trace=_want_trace, **_trace_kw,
    )

    y = np.empty((B, S, DM), dtype=np.float32)
    for b in range(B):
        acc = res.results[HPC * b]["yt"].astype(np.float32)
        for g in range(1, HPC):
            acc += res.results[HPC * b + g]["yt"].astype(np.float32)
        y[b] = acc.T
    if _want_trace:
        return y, res
    return y


# revision 25
# speedup vs baseline: 1.0105x; 1.0105x over previous
"""nn_MultiHeadAttention (B=2, S=2048, D=2048, H=16) on 8 NeuronCores.

The reference module splits heads with a plain reshape (no transpose):
    Q = (x @ Wq.T).reshape(B, H, S, Dh)
so head h attends over ROWS [128h, 128h+128) of Qmat = x @ Wq.T, with
attention position s = 16a + r mapping to (row 128h + a, feature slice
[128r, 128r+128)).  The merge DOES transpose (standard), so
    y = sum_h outh @ Wo[:, 128h:128h+128].T.

Sharding: core c handles batch b=c//4 and head-group g=c%4 (heads
4g..4g+3, i.e. Qmat/Kmat/Vmat rows [512g, 512g+512) of its batch).  Each
core computes those projection row-slices (against the FULL Wq/Wk/Wv,
streamed), causal attention in the scrambled index space, and a partial
output projection against its column slice of Wo.  The host sums the 4
partials per batch.

Attention layout per head (all on-chip, no transposes anywhere):
  k-tiles are r'-stripes {16a'+r' : a'}, q-blocks are 4 r-stripes wide.
  scoresT[k, q] tiles come out of the PE as [a', (ri, a)]; the causal mask
  within any (r', r) stripe pair is triangular in (a', a) (strict or
  inclusive depending on r' <= r), so 5 precomputed [128, 512] masks cover
  every tile.  Softmax denominators use a ones-column matmul on the PE and
  a K=1 ones-row matmul to broadcast 1/l across partitions.

Matmuls run as float32r (full-rate reduced-precision fp32) with fp32 PSUM
accumulation.
"""

import sys

try:
    import concourse.bass as bass
except ImportError:  # harness may not have the repo on PYTHONPATH
    for p in ("/root/.axon_site", "/root/.axon_site/_ro/trn_rl_repo",
              "/root/.axon_site/_ro/pypackages", "/opt/trn_rl_repo"):
        if p not in sys.path:
            sys.path.append(p)
    import concourse.bass as bass

import numpy as np

import concourse.mybir as mybir
import concourse.tile as tile
from concourse.bass_utils import run_bass_kernel_spmd

F32 = mybir.dt.float32
F32R = mybir.dt.float32r
BF16 = mybir.dt.bfloat16
DT = BF16  # on-chip matmul dtype
AF = mybir.ActivationFunctionType

B = 2
S = 2048
DM = 2048
H = 16
DH = 128
N_CORES = 8
HPC = 4                 # heads per core
DL = HPC * DH           # 512: per-core row/col slice width
P = 128
QB = 512                # q-block width = 4 r-stripes x 128 a
N_DM = DM // P          # 16 contraction tiles
NR = 16                 # r-stripes per head


def _split_multi_waits(nc):
    """This container's walrus rejects >1 sync-wait per instruction.
    Hoist extra waits onto same-engine NoOps inserted just before."""
    ctr = 0
    for f in nc.m.functions:
        for bb in f.blocks:
            insts = bb.instructions
            fixes = []
            for idx, inst in enumerate(insts):
                si = inst.sync_info
                ow = list(si.on_wait) if si and si.on_wait else []
                if len(ow) > 1:
                    fixes.append((idx, inst, ow, si))
            for idx, inst, ow, si in reversed(fixes):
                inst.sync_info = mybir.SyncInfo(on_wait=ow[-1:], on_update=si.on_update)
                for w in reversed(ow[:-1]):
                    ctr += 1
                    nop = mybir.InstNoOp(
                        name=f"I-waitsplit-{ctr}", engine=inst.engine, ins=[], outs=[]
                    )
                    nop.sync_info = mybir.SyncInfo(on_wait=[w], on_update=[])
                    nc.register_instruction(nop, overwrite=True)
                    insts.insert(idx, nop)
    return ctr


def _build_nc():
    nc = bass.Bass(target_bir_lowering=False)

    xs_d = nc.dram_tensor("xs", [DM, DL], DT, kind="ExternalInput")    # x[b,rows].T
    wqt_d = nc.dram_tensor("wqt", [DM, DM], DT, kind="ExternalInput")  # Wq.T (full)
    wkt_d = nc.dram_tensor("wkt", [DM, DM], DT, kind="ExternalInput")
    wvt_d = nc.dram_tensor("wvt", [DM, DM], DT, kind="ExternalInput")
    wot_d = nc.dram_tensor("wot", [DL, DM], DT, kind="ExternalInput")  # Wo[:,slice].T
    mask_d = nc.dram_tensor("maskc", [4, P, QB], DT, kind="ExternalInput")
    ones_d = nc.dram_tensor("ones", [P, P], DT, kind="ExternalInput")
    ident_d = nc.dram_tensor("ident", [P, P], DT, kind="ExternalInput")
    yt_d = nc.dram_tensor("yt", [DM, S], DT, kind="ExternalOutput")    # partial y[b].T

    yt_t3 = yt_d.rearrange("(o p) s -> p o s", p=P)

    with tile.TileContext(nc) as tc:
        with (
            tc.tile_pool(name="stage", bufs=4) as stage,
            tc.tile_pool(name="small", bufs=4) as small,
            tc.tile_pool(name="proj", bufs=HPC) as proj,
            tc.tile_pool(name="ps_s", bufs=3, space="PSUM") as ps_s,
            tc.tile_pool(name="ps_o", bufs=3, space="PSUM") as ps_o,
            tc.tile_pool(name="ps_l", bufs=2, space="PSUM") as ps_l,
            nc.allow_low_precision(reason="bf16 attention kernel"),
        ):
            # per-head projection tiles in [dh, a, r] layout, filled by phase A
            qt2 = [proj.tile([P, P, NR], DT, tag="qt2", name=f"qt2_{i}") for i in range(HPC)]
            kt2 = [proj.tile([P, P, NR], DT, tag="kt2", name=f"kt2_{i}") for i in range(HPC)]
            vt2 = [proj.tile([P, P, NR], DT, tag="vt2", name=f"vt2_{i}") for i in range(HPC)]

            # ---- phase A: projection row-slices straight into SBUF ----
            with (
                tc.tile_pool(name="xpool", bufs=1) as xpool,
                tc.tile_pool(name="wqk", bufs=3) as wqk,
            ):
                xs_t = xpool.tile([P, N_DM, DL], DT, tag="x")
                xs_t3 = xs_d.rearrange("(o p) s -> p o s", p=P)
                for i in range(4):
                    nc.gpsimd.dma_start(
                        xs_t[:, 4 * i:4 * (i + 1), :], xs_t3[:, 4 * i:4 * (i + 1), :]
                    )
                for w_d, dst in ((wvt_d, vt2), (wkt_d, kt2), (wqt_d, qt2)):
                    w_t3 = w_d.rearrange("(o p) d -> p o d", p=P)
                    for rt in range(NR):
                        w_t = wqk.tile([P, N_DM, P], DT, tag="wqk")
                        nc.sync.dma_start(w_t[:], w_t3[:, :, rt * P:(rt + 1) * P])
                        psum = ps_s.tile([P, QB], F32, tag="ps")
                        for dm in range(N_DM):
                            nc.tensor.matmul(
                                psum[:], lhsT=w_t[:, dm, :], rhs=xs_t[:, dm, :],
                                start=(dm == 0), stop=(dm == N_DM - 1),
                            )
                        # scatter into per-head [dh, a, r] tiles (r-strided)
                        for hl in range(HPC):
                            nc.any.tensor_copy(
                                dst[hl][:, :, rt], psum[:, hl * P:(hl + 1) * P]
                            )

            # ---- phase B: attention per head (scrambled index space) ----
            with (
                tc.tile_pool(name="bconst", bufs=1) as bconst,
                tc.tile_pool(name="hpool", bufs=2) as hpool,
                tc.tile_pool(name="atpool", bufs=6) as atpool,
                tc.tile_pool(name="attt", bufs=HPC) as attt_pool,
            ):
                ones_t = bconst.tile([P, P], DT, tag="ones")
                nc.sync.dma_start(ones_t[:], ones_d[:])
                mask_t = bconst.tile([P, 4, QB], DT, tag="mask")
                nc.sync.dma_start(mask_t[:], mask_d.rearrange("c p q -> p c q"))
                ident_t = bconst.tile([P, P], DT, tag="ident")
                nc.sync.dma_start(ident_t[:], ident_d[:])

                att_tiles = []
                for hl in range(HPC):
                    # vk: k-major V tiles via PE transpose, partition=(a'',r')
                    vk_h = hpool.tile([P, NR, P], DT, tag="v")    # [(a'' r'), m, dh]

                    def emit_vk(m, hl=hl, vk_h=vk_h):
                        ps_t = ps_o.tile([P, P], DT, tag="po")
                        nc.tensor.transpose(
                            ps_t[:], vt2[hl][:, 8 * m:8 * (m + 1), :], ident_t[:]
                        )
                        nc.any.tensor_copy(vk_h[:, m, :], ps_t[:])

                    att_h = attt_pool.tile([P, P, NR], DT, tag="attT")  # [dh, a, r]
                    att_tiles.append(att_h)

                    for qb in range(4):
                        a0 = 32 * qb
                        nk = 4 * (qb + 1)   # k-octets 0..nk-1
                        for m in range(4 * qb, nk):
                            emit_vk(m)
                        psum_o = ps_o.tile([P, QB], F32, tag="po")
                        psum_l = ps_l.tile([P, QB], F32, tag="pl")
                        ats = [None] * nk

                        def emit_scores(m):
                            psum_s = ps_s.tile([P, QB], F32, tag="ps")
                            nc.tensor.matmul(
                                psum_s[:],
                                lhsT=kt2[hl][:, 8 * m:8 * (m + 1), :],
                                rhs=qt2[hl][:, a0:a0 + 32, :],
                                start=True, stop=True,
                            )
                            at = atpool.tile([P, QB], DT, tag="at")
                            nc.scalar.activation(at[:], psum_s[:], AF.Exp, scale=1.0 / DH)
                            if m >= 4 * qb:
                                nc.vector.tensor_mul(at[:], at[:], mask_t[:, m - 4 * qb, :])
                            ats[m] = at

                        def emit_ov(m):
                            nc.tensor.matmul(
                                psum_o[:],
                                lhsT=vk_h[:, m, :], rhs=ats[m][:],
                                start=(m == 0), stop=(m == nk - 1),
                            )
                            nc.tensor.matmul(
                                psum_l[:],
                                lhsT=ones_t[:, :], rhs=ats[m][:],
                                start=(m == 0), stop=(m == nk - 1),
                            )

                        DEPTH = 2
                        for m in range(nk):
                            emit_scores(m)
                            if m >= DEPTH:
                                emit_ov(m - DEPTH)
                        for m in range(nk - DEPTH, nk):
                            emit_ov(m)

                        # normalize: att = psum_o * (1/l)
                        rcb = small.tile([P, QB], F32, tag="rcb")
                        nc.vector.reciprocal(rcb[:], psum_l[:])
                        nc.vector.tensor_mul(
                            att_h[:, a0:a0 + 32, :],
                            psum_o[:].rearrange("p (a r) -> p a r", a=32),
                            rcb[:].rearrange("p (a r) -> p a r", a=32),
                        )

                # ---- phase C: partial output projection yT = WoT.T @ attT ----
                with tc.tile_pool(name="wop", bufs=1) as wop:
                    wot_t = wop.tile([P, HPC, DM], DT, tag="wo")
                    nc.sync.dma_start(
                        wot_t[:],
                        wot_d.rearrange("(hl p) d -> p hl d", p=P),
                    )
                    att_flat = [
                        t[:].rearrange("p a r -> p (a r)") for t in att_tiles
                    ]
                    for ot in range(N_DM):
                        for sb in range(4):
                            psum = ps_s.tile([P, QB], F32, tag="ps")
                            for hl in range(HPC):
                                nc.tensor.matmul(
                                    psum[:],
                                    lhsT=wot_t[:, hl, ot * P:(ot + 1) * P],
                                    rhs=att_flat[hl][:, sb * QB:(sb + 1) * QB],
                                    start=(hl == 0), stop=(hl == HPC - 1),
                                )
                            st = stage.tile([P, QB], DT, tag="ystage")
                            nc.any.tensor_copy(st[:], psum[:])
                            nc.gpsimd.dma_start(yt_t3[:, ot, sb * QB:(sb + 1) * QB], st[:])

    _split_multi_waits(nc)
    return nc


_NC = None


def _make_masks():
    # a-blocked causal masks for diagonal tiles, (a-outer, r-inner) order:
    # k partition index p = a''*16 + r';  q column index j = a_rel*16 + r
    # allow k <= q:  16*(8*mi + a'') + r'  <=  16*a_rel + r
    k_lin = (16 * np.arange(8)[:, None] + np.arange(NR)[None, :]).reshape(-1)   # 128
    q_lin = (16 * np.arange(32)[:, None] + np.arange(NR)[None, :]).reshape(-1)  # 512
    out = np.empty((4, P, QB), dtype=np.float32)
    for mi in range(4):
        out[mi] = ((k_lin[:, None] + 128 * mi) <= q_lin[None, :]).astype(np.float32)
    return out


def kernel(x, Wq, Wk, Wv, Wo, _want_trace=False, **_trace_kw):
    global _NC
    if _NC is None:
        _NC = _build_nc()
    nc = _NC

    import ml_dtypes
    bf16 = ml_dtypes.bfloat16

    x = np.asarray(x, dtype=np.float32)
    wqt = np.ascontiguousarray(np.asarray(Wq, dtype=np.float32).T).astype(bf16)
    wkt = np.ascontiguousarray(np.asarray(Wk, dtype=np.float32).T).astype(bf16)
    wvt = np.ascontiguousarray(np.asarray(Wv, dtype=np.float32).T).astype(bf16)
    Wo = np.asarray(Wo, dtype=np.float32)
    masks = _make_masks().astype(bf16)
    ones = np.ones((P, P), dtype=bf16)
    ident = np.eye(P, dtype=np.float32).astype(bf16)

    in_maps = []
    for c in range(N_CORES):
        b, g = divmod(c, HPC)
        sl = slice(g * DL, (g + 1) * DL)
        in_maps.append({
            "xs": np.ascontiguousarray(x[b, sl, :].T).astype(bf16),
            "wqt": wqt,
            "wkt": wkt,
            "wvt": wvt,
            "wot": np.ascontiguousarray(Wo[:, sl].T).astype(bf16),
            "maskc": masks,
            "ones": ones,
            "ident": ident,
        })

    res = run_bass_kernel_spmd(
        nc, in_maps, list(range(N_CORES)),
        trace=_want_trace, **_trace_kw,
    )

    y = np.empty((B, S, DM), dtype=np.float32)
    for b in range(B):
        acc = res.results[HPC * b]["yt"].astype(np.float32)
        for g in range(1, HPC):
            acc += res.results[HPC * b + g]["yt"].astype(np.float32)
        y[b] = acc.T
    if _want_trace:
        return y, res
    return y
